# revision 13
# baseline (speedup 1.0000x reference)
"""Trainium2 Bass kernel for BCNLayer (3x3 per-position-weighted spatial
shift conv over a 128x128 grid + sigmoid).

y[yo,xo,b] = sigmoid( sum_{dy,dx in {-1,0,1}} w[dy+1,dx+1,(yo-dy)*128+(xo-dx)]
                      * x[(yo-dy)*128+(xo-dx), b] )   (zero outside the grid)

Formulation: for each output row yo, y_row[yo] = sigmoid( sum_{yi in
{yo-1,yo,yo+1}} T[dy,yi].T @ x_row[yi] ) where T[dy,yi] is a 128x128
tridiagonal matrix holding the three dx weight vectors of input row yi on
its diagonals (dy = yo-yi).  T matrices are built on-chip in fp16 from an
SBUF weight image: per dy a gpsimd affine_select places the j=0 diagonal
(zero-filling the rest) and two DVE predicated copies place j=1,2; a
130-wide buffer with the matmul reading cols 1:129 makes the x-boundary
masking fall out of the padding columns.

Sharding: by GRID ROWS, not batch -- core c owns output rows
[16c, 16c+16) with the full 4096-sample batch.  Each T block then feeds
4096 moving columns instead of 512, so the (per-lane-sweep-heavy) T
build shrinks 8x to ~13us and hides completely under the DMA stream.
Each core's x input is its 16 rows plus a one-row halo on each side,
zero-padded at the grid edges host-side, which makes every output row
see exactly 3 input rows (no boundary special cases on-chip).

The moving operand is fp16 (SWDGE DMAs cast f32 -> fp16 inline at line
rate); fp16 weights keep the 1-col/cycle PE path with no rounding pass.
Sigmoid is batched: 4 PSUM banks (half a row's batch) per ACT
instruction.  Outputs stage 2 full rows and store as 4 MiB fp16 DMAs
alternating between the two HWDGE rings (SP + ACT) so the ~2us HBM
write-completion receipts pipeline.
"""

import numpy as np

H = 128
W = 128
HW = H * W
B = 4096
NCORES = 8
YR = H // NCORES  # 16 output grid rows per core
XR = YR + 2  # input rows incl one halo row each side
CL = 2  # x rows per load DMA (2 * 128 part * 4096 * 4B = 4 MiB)
G = 6  # yi rows per T-group tile (3 groups cover the 18 local rows)
NB = B // 512  # 8 psum-bank-sized batch slices per row

_CACHE = {}


def _make_tile_context_cls():
    import concourse.tile as tile
    import bass_rust

    class SplitDrainTileContext(tile.TileContext):
        """The walrus build in this container accepts at most one sem-wait
        per instruction; Tile freely emits several (e.g. a matmul waiting
        on both operand DMA lanes).  Split the extras onto single-wait
        nops emitted just before the instruction on the same engine."""

        def _add_instruction(self, inst):
            from concourse import mybir as _mybir

            si = inst.sync_info
            if si is not None and si.on_wait and len(si.on_wait) > 1:
                waits = list(si.on_wait)
                si.on_wait = [waits[-1]]
                for w in waits[:-1]:
                    nop = _mybir.InstNoOp(
                        name=self.nc.get_next_instruction_name(),
                        ins=[],
                        outs=[],
                    )
                    nop.engine = inst.engine
                    nop.sync_info = _mybir.SyncInfo(on_wait=[w], on_update=[])
                    super()._add_instruction(nop)
            super()._add_instruction(inst)

        def _drain_and_barrier(self, tick_clock, wait_clock):
            collector = self.nc.sync.nop(nofuse=True, hint="tail_waits")
            wait_clock.add_sem_waits(
                collector.ins,
                bass_rust.ScopedClock({None: tick_clock.global_clock}),
            )
            si = collector.ins.sync_info
            waits = list(si.on_wait) if si is not None and si.on_wait else []
            if len(waits) > 1:
                si.on_wait = [waits[0]]
                from concourse import mybir as _mybir

                for w in waits[1:]:
                    n = self.nc.sync.nop(nofuse=True, hint="tail_waits")
                    n.ins.sync_info = _mybir.SyncInfo(on_wait=[w], on_update=[])
            self.nc.sync.drain()
            self.nc.all_engine_barrier()
            assert self.sems is not None
            popped = self.nc._tile_sem_poison_stack.pop()
            assert popped is self._sem_poison
            self.nc.clear_and_free_semaphores(
                list(self.sems.allocated().values())
            )
            self.nc.all_engine_barrier()

    return SplitDrainTileContext


def _build_nc(repeat=1):
    import concourse.bass as bass
    import concourse.tile as tile
    import concourse.mybir as mybir
    from concourse.ap import AP

    tile_context_cls = _make_tile_context_cls()
    f32 = mybir.dt.float32
    f16 = mybir.dt.float16
    nc = bass.Bass("TRN2", target_bir_lowering=False, debug=False)
    # per-core x slice: 18 rows (16 owned + halo), zero-padded at edges
    x = nc.dram_tensor("x", [XR * 128, B], f32, kind="ExternalInput")
    # wsb[xi, (i, l, j)] = w[i, j, yi_abs(l)*128+xi] for local rows l
    wsb_d = nc.dram_tensor("wsb", [128, 3 * XR * 3], f16,
                           kind="ExternalInput")
    # y stored fp16 (sigmoid output in [0,1]: adds <= ~2.4e-4 abs error)
    # and upcast to f32 on the host -- halves the output DMA traffic
    y = nc.dram_tensor("y", [YR * 128, B], f16, kind="ExternalOutput")

    NCH = XR // CL  # 9 x row-chunks
    NGR = XR // G  # 3 T-groups
    TW = 130  # T used width: col c = xi + j, lhsT reads cols 1:129
    # T stored stride: != TW so strided APs never dim-merge, and EVEN so
    # every fp16 block starts 4B-aligned (packed-pair DVE modes need it)
    TWS = 132
    WPITCH = 3 * XR * 3  # wsb row length
    TPITCH = 3 * G * TWS  # T-group tile row length

    with tile_context_cls(nc) as tc:
        with (
            tc.tile_pool(name="cn", bufs=1) as cpool,
            tc.tile_pool(name="xp", bufs=4) as xpool,
            tc.tile_pool(name="op", bufs=2) as opool,
            tc.tile_pool(name="ps", bufs=2, space="PSUM") as ppool,
        ):
            # one-time: weight image, one-hot diagonal masks
            wsb = cpool.tile([128, WPITCH], f16)
            nc.sync.dma_start(out=wsb[:], in_=wsb_d.ap())
            i16 = mybir.dt.int16
            ones = cpool.tile([128, TW], i16)
            nc.gpsimd.memset(ones[:], 1)
            # mask rows stride TWS (even) so each j-row starts 4B-aligned
            masks = cpool.tile([128, 3, TWS], i16)
            for j in range(1, 3):
                # D_j[xi, c] = 1 where c - xi - j == 0
                nc.gpsimd.affine_select(
                    masks[:, j, 0:TW], ones[:],
                    pattern=[[1, TW]], base=-j, channel_multiplier=-1,
                    compare_op=mybir.AluOpType.is_equal, fill=0,
                )
            # T-group tiles live for the whole pass (all 18 rows resident)
            tgroups = [
                cpool.tile([128, 3, G, TWS], f16, name=f"tg{_g}")
                for _g in range(NGR)
            ]

            xt = {}
            built = set()

            def load_chunk(c):  # noqa: closure rebound per repeat
                if c in xt or c >= NCH:
                    return
                t = xpool.tile([128, CL, B], f16, tag="xchunk")
                # 4 MiB reads; SWDGE (gpsimd) casts f32 -> fp16 inline,
                # which the 16-bit PE path needs of the moving side
                src = AP(
                    x.ap().tensor,
                    c * CL * 128 * B,
                    [[B, 128], [128 * B, CL], [1, B]],
                )
                nc.gpsimd.dma_start(out=t[:], in_=src)
                xt[c] = t

            def build_group(g):
                if g in built or g >= NGR:
                    return
                built.add(g)
                t = tgroups[g]
                ta = t[:]
                wv = wsb[:]
                for i in range(3):
                    out_i = AP(ta.tensor, ta.offset + i * G * TWS,
                               [[TPITCH, 128], [TWS, G], [1, TW]])

                    def wb(j):
                        return AP(wv.tensor,
                                  wv.offset + i * XR * 3 + g * G * 3 + j,
                                  [[WPITCH, 128], [3, G], [0, TW]])

                    # j=0 placed directly by an affine select (c == xi):
                    # zero-fills the whole block and writes the diagonal in
                    # one gpsimd pass, leaving DVE only the two predicated
                    # copies per dy
                    nc.gpsimd.affine_select(
                        out_i, wb(0),
                        pattern=[[0, G], [1, TW]], base=0,
                        channel_multiplier=-1,
                        compare_op=mybir.AluOpType.is_equal, fill=0.0,
                    )
                    for j in range(1, 3):
                        ma = masks[:, j, :]
                        mb = AP(ma.tensor, ma.offset,
                                [[3 * TWS, 128], [0, G], [1, TW]])
                        nc.vector.copy_predicated(out_i, mb, wb(j))

            for _rep in range(repeat):
              if _rep:
                  xt.clear()
                  built = set()
              # prime: first chunks + the whole (tiny) T build; MMs start
              # as soon as group 0 and chunk 0 land
              for _c in range(3):
                  load_chunk(_c)
              for _g in range(NGR):
                  build_group(_g)

              ystage = None
              for yo in range(YR):
                  load_chunk((yo + 2) // CL + 1)
                  if yo % 2 == 0:
                      ystage = opool.tile([128, 2, B], f16, tag="yst")
                  for half in range(2):
                      pt = ppool.tile([128, 4, 512], f32, tag="psum")
                      # local input rows l = yo, yo+1, yo+2 (dy = yo+1-l);
                      # the halo padding makes all three always valid
                      for k, l in enumerate((yo, yo + 1, yo + 2)):
                          i_dy = yo + 2 - l
                          tg = tgroups[l // G][:]
                          lo = (i_dy * G + (l % G)) * TWS + 1
                          lhsT = AP(tg.tensor, tg.offset + lo,
                                    [[TPITCH, 128], [1, 128]])
                          xc = xt[l // CL][:]
                          # 4 bank-sized batch slices share this lhsT, so
                          # the PE streams 2048 columns per weight load
                          for bk in range(4):
                              rhs = AP(
                                  xc.tensor,
                                  xc.offset + (l % CL) * B
                                  + half * 2048 + bk * 512,
                                  [[CL * B, 128], [1, 512]],
                              )
                              nc.tensor.matmul(
                                  pt[:, bk, :],
                                  lhsT,
                                  rhs,
                                  start=(k == 0),
                                  stop=(k == 2),
                              )
                      # one ACT instruction reads the whole 4-bank PSUM
                      # tile (2048 f32) and writes fp16: amortizes the
                      # per-instruction ACT overhead 4x
                      ys = ystage[:]
                      dsl = AP(
                          ys.tensor,
                          ys.offset + (yo % 2) * B + half * 2048,
                          [[2 * B, 128], [1, 2048]],
                      )
                      pf = pt[:]
                      nc.scalar.activation(
                          dsl,
                          AP(pf.tensor, pf.offset, [[2048, 128], [1, 2048]]),
                          mybir.ActivationFunctionType.Sigmoid,
                      )
                  if yo % 2 == 1:
                      dst = AP(
                          y.ap().tensor,
                          (yo - 1) * 128 * B,
                          [[B, 128], [128 * B, 2], [1, B]],
                      )
                      # 4 MiB stores alternating across BOTH HWDGE rings
                      # (SP + ACT): the ~2us HBM write-completion receipt
                      # serializes per ring, so two rings pipeline it
                      eng = nc.scalar if (yo // 2) % 2 == 0 else nc.sync
                      eng.dma_start(out=dst, in_=ystage[:])
    return nc


def get_nc():
    if "nc" not in _CACHE:
        _CACHE["nc"] = _build_nc()
    return _CACHE["nc"]


def make_in_maps(x, w):
    x = np.asarray(x, dtype=np.float32)
    # x with a zero halo row of the 128x128 grid on each side
    xp = np.zeros((HW + 2 * 128, B), np.float32)
    xp[128 : 128 + HW] = x
    # weight image wg[xi, i, yi, j] = w[i, j, yi*128+xi], halo-padded in yi
    wg = (
        np.asarray(w, dtype=np.float32)
        .reshape(3, 3, H, W)
        .transpose(3, 0, 2, 1)
        .astype(np.float16)
    )  # [xi, i, yi, j]
    wgp = np.zeros((128, 3, H + 2, 3), np.float16)
    wgp[:, :, 1 : 1 + H] = wg
    in_maps = []
    for c in range(NCORES):
        xs = np.ascontiguousarray(xp[c * YR * 128 : (c * YR + XR) * 128])
        ws = np.ascontiguousarray(
            wgp[:, :, c * YR : c * YR + XR].reshape(128, 3 * XR * 3)
        )
        in_maps.append({"x": xs, "wsb": ws})
    return in_maps


def kernel(x: np.ndarray, w: np.ndarray) -> np.ndarray:
    import time as _time

    from concourse.bass_utils import run_bass_kernel_spmd

    in_maps = make_in_maps(x, w)
    nc = get_nc()
    # The compile hook / remote execution path occasionally fails
    # transiently (observed: a flaky walrus invocation and a recoverable
    # NRT exec error); retry a few times before giving up.
    last_exc = None
    for attempt in range(4):
        try:
            res = run_bass_kernel_spmd(
                nc, in_maps, list(range(NCORES))
            ).results
            break
        except Exception as exc:  # noqa: BLE001
            last_exc = exc
            _time.sleep(2.0 * (attempt + 1))
    else:
        raise last_exc
    return np.ascontiguousarray(
        np.concatenate([res[i]["y"] for i in range(NCORES)], axis=0),
        dtype=np.float32,
    )


# revision 14
# speedup vs baseline: 1.1061x; 1.1061x over previous
"""Trainium2 Bass kernel for BCNLayer (3x3 per-position-weighted spatial
shift conv over a 128x128 grid + sigmoid).

y[yo,xo,b] = sigmoid( sum_{dy,dx in {-1,0,1}} w[dy+1,dx+1,(yo-dy)*128+(xo-dx)]
                      * x[(yo-dy)*128+(xo-dx), b] )   (zero outside the grid)

Formulation: for each output row yo, y_row[yo] = sigmoid( sum_{yi in
{yo-1,yo,yo+1}} T[dy,yi].T @ x_row[yi] ) where T[dy,yi] is a 128x128
tridiagonal matrix holding the three dx weight vectors of input row yi on
its diagonals (dy = yo-yi).  T matrices are built on-chip in fp16 from an
SBUF weight image: per dy a gpsimd affine_select places the j=0 diagonal
(zero-filling the rest) and two DVE predicated copies place j=1,2; a
130-wide buffer with the matmul reading cols 1:129 makes the x-boundary
masking fall out of the padding columns.

Sharding: a 4x2 grid -- 4 row-blocks of 32 output grid rows x 2 batch
halves of 2048.  Sharding mostly by rows lets each T block feed 2048
moving columns, so the (per-lane-sweep-heavy) T build shrinks to ~28us
and hides under the DMA stream; 32-row blocks keep the one-row halo at
6.25% of the x read.  Halo rows are zero-padded at the grid edges
host-side, which makes every output row see exactly 3 input rows (no
boundary special cases on-chip).

The moving operand is fp16 (SWDGE DMAs cast f32 -> fp16 inline at line
rate); fp16 weights keep the 1-col/cycle PE path with no rounding pass.
Sigmoid is batched: one ACT instruction reads a full row's 4 PSUM banks
(2048 f32).  Outputs stage 4 rows and store as 2 MiB fp16 DMAs
alternating between the two HWDGE rings (SP + ACT) so the ~2us HBM
write-completion receipts pipeline.
"""

import numpy as np

H = 128
W = 128
HW = H * W
B = 4096
NCORES = 8
NRB = 4  # row blocks
NBB = 2  # batch blocks
YR = H // NRB  # 32 output grid rows per core
XR = YR + 2  # input rows incl one halo row each side
BCC = B // NBB  # 2048 batch columns per core
CL = 2  # x rows per load DMA (2 * 128 part * 2048 * 4B = 2 MiB)
G = 17  # yi rows per T-group tile (2 groups cover the 34 local rows)
SRW = 4  # y rows per store DMA (4 * 128 * 2048 * 2B = 2 MiB)

_CACHE = {}


def _make_tile_context_cls():
    import concourse.tile as tile
    import bass_rust

    class SplitDrainTileContext(tile.TileContext):
        """The walrus build in this container accepts at most one sem-wait
        per instruction; Tile freely emits several (e.g. a matmul waiting
        on both operand DMA lanes).  Split the extras onto single-wait
        nops emitted just before the instruction on the same engine."""

        def _add_instruction(self, inst):
            from concourse import mybir as _mybir

            si = inst.sync_info
            if si is not None and si.on_wait and len(si.on_wait) > 1:
                waits = list(si.on_wait)
                si.on_wait = [waits[-1]]
                for w in waits[:-1]:
                    nop = _mybir.InstNoOp(
                        name=self.nc.get_next_instruction_name(),
                        ins=[],
                        outs=[],
                    )
                    nop.engine = inst.engine
                    nop.sync_info = _mybir.SyncInfo(on_wait=[w], on_update=[])
                    super()._add_instruction(nop)
            super()._add_instruction(inst)

        def _drain_and_barrier(self, tick_clock, wait_clock):
            collector = self.nc.sync.nop(nofuse=True, hint="tail_waits")
            wait_clock.add_sem_waits(
                collector.ins,
                bass_rust.ScopedClock({None: tick_clock.global_clock}),
            )
            si = collector.ins.sync_info
            waits = list(si.on_wait) if si is not None and si.on_wait else []
            if len(waits) > 1:
                si.on_wait = [waits[0]]
                from concourse import mybir as _mybir

                for w in waits[1:]:
                    n = self.nc.sync.nop(nofuse=True, hint="tail_waits")
                    n.ins.sync_info = _mybir.SyncInfo(on_wait=[w], on_update=[])
            self.nc.sync.drain()
            self.nc.all_engine_barrier()
            assert self.sems is not None
            popped = self.nc._tile_sem_poison_stack.pop()
            assert popped is self._sem_poison
            self.nc.clear_and_free_semaphores(
                list(self.sems.allocated().values())
            )
            self.nc.all_engine_barrier()

    return SplitDrainTileContext


def _build_nc(repeat=1):
    import concourse.bass as bass
    import concourse.tile as tile
    import concourse.mybir as mybir
    from concourse.ap import AP

    tile_context_cls = _make_tile_context_cls()
    f32 = mybir.dt.float32
    f16 = mybir.dt.float16
    nc = bass.Bass("TRN2", target_bir_lowering=False, debug=False)
    # per-core x slice: 34 rows (32 owned + halo), zero-padded at edges
    x = nc.dram_tensor("x", [XR * 128, BCC], f32, kind="ExternalInput")
    # wsb[xi, (i, l, j)] = w[i, j, yi_abs(l)*128+xi] for local rows l
    wsb_d = nc.dram_tensor("wsb", [128, 3 * XR * 3], f16,
                           kind="ExternalInput")
    # y stored fp16 (sigmoid output in [0,1]: adds <= ~2.4e-4 abs error)
    # and upcast to f32 on the host -- halves the output DMA traffic
    y = nc.dram_tensor("y", [YR * 128, BCC], f16, kind="ExternalOutput")

    NCH = XR // CL  # 17 x row-chunks
    NGR = XR // G  # 2 T-groups
    TW = 130  # T used width: col c = xi + j, lhsT reads cols 1:129
    # T stored stride: != TW so strided APs never dim-merge, and EVEN so
    # every fp16 block starts 4B-aligned (packed-pair DVE modes need it)
    TWS = 132
    WPITCH = 3 * XR * 3  # wsb row length
    TPITCH = 3 * G * TWS  # T-group tile row length

    with tile_context_cls(nc) as tc:
        with (
            tc.tile_pool(name="cn", bufs=1) as cpool,
            tc.tile_pool(name="xp", bufs=5) as xpool,
            tc.tile_pool(name="op", bufs=3) as opool,
            tc.tile_pool(name="ps", bufs=2, space="PSUM") as ppool,
        ):
            # one-time: weight image, one-hot diagonal masks
            wsb = cpool.tile([128, WPITCH], f16)
            nc.sync.dma_start(out=wsb[:], in_=wsb_d.ap())
            i16 = mybir.dt.int16
            ones = cpool.tile([128, TW], i16)
            nc.gpsimd.memset(ones[:], 1)
            # mask rows stride TWS (even) so each j-row starts 4B-aligned
            masks = cpool.tile([128, 3, TWS], i16)
            for j in range(1, 3):
                # D_j[xi, c] = 1 where c - xi - j == 0
                nc.gpsimd.affine_select(
                    masks[:, j, 0:TW], ones[:],
                    pattern=[[1, TW]], base=-j, channel_multiplier=-1,
                    compare_op=mybir.AluOpType.is_equal, fill=0,
                )
            # T-group tiles live for the whole pass (all 34 rows resident)
            tgroups = [
                cpool.tile([128, 3, G, TWS], f16, name=f"tg{_g}")
                for _g in range(NGR)
            ]

            xt = {}
            built = set()

            def load_chunk(c):  # noqa: closure rebound per repeat
                if c in xt or c >= NCH:
                    return
                t = xpool.tile([128, CL, BCC], f16, tag="xchunk")
                # 2 MiB reads; SWDGE (gpsimd) casts f32 -> fp16 inline,
                # which the 16-bit PE path needs of the moving side
                src = AP(
                    x.ap().tensor,
                    c * CL * 128 * BCC,
                    [[BCC, 128], [128 * BCC, CL], [1, BCC]],
                )
                nc.gpsimd.dma_start(out=t[:], in_=src)
                xt[c] = t

            def build_group(g):
                if g in built or g >= NGR:
                    return
                built.add(g)
                t = tgroups[g]
                ta = t[:]
                wv = wsb[:]
                for i in range(3):
                    out_i = AP(ta.tensor, ta.offset + i * G * TWS,
                               [[TPITCH, 128], [TWS, G], [1, TW]])

                    def wb(j):
                        return AP(wv.tensor,
                                  wv.offset + i * XR * 3 + g * G * 3 + j,
                                  [[WPITCH, 128], [3, G], [0, TW]])

                    # j=0 placed directly by an affine select (c == xi):
                    # zero-fills the whole block and writes the diagonal in
                    # one gpsimd pass, leaving DVE only the two predicated
                    # copies per dy
                    nc.gpsimd.affine_select(
                        out_i, wb(0),
                        pattern=[[0, G], [1, TW]], base=0,
                        channel_multiplier=-1,
                        compare_op=mybir.AluOpType.is_equal, fill=0.0,
                    )
                    for j in range(1, 3):
                        ma = masks[:, j, :]
                        mb = AP(ma.tensor, ma.offset,
                                [[3 * TWS, 128], [0, G], [1, TW]])
                        nc.vector.copy_predicated(out_i, mb, wb(j))

            for _rep in range(repeat):
              if _rep:
                  xt.clear()
                  built = set()
              # prime: first chunks + the whole (small) T build; MMs start
              # as soon as group 0 and chunk 0 land
              for _c in range(3):
                  load_chunk(_c)
              for _g in range(NGR):
                  build_group(_g)

              ystage = None
              for yo in range(YR):
                  load_chunk((yo + 2) // CL + 1)
                  load_chunk((yo + 2) // CL + 2)
                  if yo % SRW == 0:
                      ystage = opool.tile([128, SRW, BCC], f16, tag="yst")
                  pt = ppool.tile([128, 4, 512], f32, tag="psum")
                  # local input rows l = yo, yo+1, yo+2 (dy = yo+1-l);
                  # the halo padding makes all three always valid
                  for k, l in enumerate((yo, yo + 1, yo + 2)):
                      i_dy = yo + 2 - l
                      tg = tgroups[l // G][:]
                      lo = (i_dy * G + (l % G)) * TWS + 1
                      lhsT = AP(tg.tensor, tg.offset + lo,
                                [[TPITCH, 128], [1, 128]])
                      xc = xt[l // CL][:]
                      # 4 bank-sized batch slices share this lhsT, so the
                      # PE streams 2048 columns per weight load
                      for bk in range(4):
                          rhs = AP(
                              xc.tensor,
                              xc.offset + (l % CL) * BCC + bk * 512,
                              [[CL * BCC, 128], [1, 512]],
                          )
                          nc.tensor.matmul(
                              pt[:, bk, :],
                              lhsT,
                              rhs,
                              start=(k == 0),
                              stop=(k == 2),
                          )
                  # one ACT instruction reads the whole 4-bank PSUM tile
                  # (2048 f32) and writes a full row of fp16: amortizes
                  # the per-instruction ACT overhead 4x
                  ys = ystage[:]
                  dsl = AP(
                      ys.tensor,
                      ys.offset + (yo % SRW) * BCC,
                      [[SRW * BCC, 128], [1, BCC]],
                  )
                  pf = pt[:]
                  nc.scalar.activation(
                      dsl,
                      AP(pf.tensor, pf.offset, [[2048, 128], [1, 2048]]),
                      mybir.ActivationFunctionType.Sigmoid,
                  )
                  if yo % SRW == SRW - 1:
                      dst = AP(
                          y.ap().tensor,
                          (yo - (SRW - 1)) * 128 * BCC,
                          [[BCC, 128], [128 * BCC, SRW], [1, BCC]],
                      )
                      # 2 MiB stores alternating across BOTH HWDGE rings
                      # (SP + ACT): the ~2us HBM write-completion receipt
                      # serializes per ring, so two rings pipeline it
                      eng = nc.scalar if (yo // SRW) % 2 == 0 else nc.sync
                      eng.dma_start(out=dst, in_=ystage[:])
    return nc


def get_nc():
    if "nc" not in _CACHE:
        _CACHE["nc"] = _build_nc()
    return _CACHE["nc"]


def make_in_maps(x, w):
    x = np.asarray(x, dtype=np.float32)
    # x with a zero halo row of the 128x128 grid on each side
    xp = np.zeros((HW + 2 * 128, B), np.float32)
    xp[128 : 128 + HW] = x
    # weight image wg[xi, i, yi, j] = w[i, j, yi*128+xi], halo-padded in yi
    wg = (
        np.asarray(w, dtype=np.float32)
        .reshape(3, 3, H, W)
        .transpose(3, 0, 2, 1)
        .astype(np.float16)
    )  # [xi, i, yi, j]
    wgp = np.zeros((128, 3, H + 2, 3), np.float16)
    wgp[:, :, 1 : 1 + H] = wg
    in_maps = []
    for c in range(NCORES):
        rb, bb = c // NBB, c % NBB
        xs = np.ascontiguousarray(
            xp[rb * YR * 128 : (rb * YR + XR) * 128,
               bb * BCC : (bb + 1) * BCC]
        )
        ws = np.ascontiguousarray(
            wgp[:, :, rb * YR : rb * YR + XR].reshape(128, 3 * XR * 3)
        )
        in_maps.append({"x": xs, "wsb": ws})
    return in_maps


def kernel(x: np.ndarray, w: np.ndarray) -> np.ndarray:
    import time as _time

    from concourse.bass_utils import run_bass_kernel_spmd

    in_maps = make_in_maps(x, w)
    nc = get_nc()
    # The compile hook / remote execution path occasionally fails
    # transiently (observed: a flaky walrus invocation and a recoverable
    # NRT exec error); retry a few times before giving up.
    last_exc = None
    for attempt in range(4):
        try:
            res = run_bass_kernel_spmd(
                nc, in_maps, list(range(NCORES))
            ).results
            break
        except Exception as exc:  # noqa: BLE001
            last_exc = exc
            _time.sleep(2.0 * (attempt + 1))
    else:
        raise last_exc
    out = np.empty((HW, B), np.float32)
    for c in range(NCORES):
        rb, bb = c // NBB, c % NBB
        out[rb * YR * 128 : (rb + 1) * YR * 128,
            bb * BCC : (bb + 1) * BCC] = res[c]["y"]
    return out


# revision 16
# speedup vs baseline: 1.1876x; 1.0737x over previous
"""Trainium2 Bass kernel for BCNLayer (3x3 per-position-weighted spatial
shift conv over a 128x128 grid + sigmoid).

y[yo,xo,b] = sigmoid( sum_{dy,dx in {-1,0,1}} w[dy+1,dx+1,(yo-dy)*128+(xo-dx)]
                      * x[(yo-dy)*128+(xo-dx), b] )   (zero outside the grid)

Formulation: for each output row yo, y_row[yo] = sigmoid( sum_{yi in
{yo-1,yo,yo+1}} T[dy,yi].T @ x_row[yi] ) where T[dy,yi] is a 128x128
tridiagonal matrix holding the three dx weight vectors of input row yi on
its diagonals (dy = yo-yi).  T matrices are built on-chip in fp16 from an
SBUF weight image: per dy a gpsimd affine_select places the j=0 diagonal
(zero-filling the rest) and two DVE predicated copies place j=1,2; a
130-wide buffer with the matmul reading cols 1:129 makes the x-boundary
masking fall out of the padding columns.

Sharding: a 4x2 grid -- 4 row-blocks of 32 output grid rows x 2 batch
halves of 2048.  Sharding mostly by rows lets each T block feed 2048
moving columns, so the (per-lane-sweep-heavy) T build shrinks to ~28us
and hides under the DMA stream; 32-row blocks keep the one-row halo at
6.25% of the x read.  Halo rows are zero-padded at the grid edges
host-side, which makes every output row see exactly 3 input rows (no
boundary special cases on-chip).

The moving operand is fp16 (SWDGE DMAs cast f32 -> fp16 inline at line
rate); fp16 weights keep the 1-col/cycle PE path with no rounding pass.
Sigmoid is batched: one ACT instruction reads a full row's 4 PSUM banks
(2048 f32).  Outputs stage 4 rows and store as 2 MiB fp16 DMAs
alternating between the two HWDGE rings (SP + ACT) so the ~2us HBM
write-completion receipts pipeline.
"""

import numpy as np

H = 128
W = 128
HW = H * W
B = 4096
NCORES = 8
NRB = 4  # row blocks
NBB = 2  # batch blocks
YR = H // NRB  # 32 output grid rows per core
XR = YR + 2  # input rows incl one halo row each side
BCC = B // NBB  # 2048 batch columns per core
CL = 2  # x rows per load DMA (2 * 128 part * 2048 * 4B = 2 MiB)
G = 17  # yi rows per T-group tile (2 groups cover the 34 local rows)
SRW = 4  # y rows per store DMA (4 * 128 * 2048 * 2B = 2 MiB)

_CACHE = {}


def _make_tile_context_cls():
    import concourse.tile as tile
    import bass_rust

    class SplitDrainTileContext(tile.TileContext):
        """The walrus build in this container accepts at most one sem-wait
        per instruction; Tile freely emits several (e.g. a matmul waiting
        on both operand DMA lanes).  Split the extras onto single-wait
        nops emitted just before the instruction on the same engine."""

        def _add_instruction(self, inst):
            from concourse import mybir as _mybir

            si = inst.sync_info
            if si is not None and si.on_wait and len(si.on_wait) > 1:
                waits = list(si.on_wait)
                si.on_wait = [waits[-1]]
                for w in waits[:-1]:
                    nop = _mybir.InstNoOp(
                        name=self.nc.get_next_instruction_name(),
                        ins=[],
                        outs=[],
                    )
                    nop.engine = inst.engine
                    nop.sync_info = _mybir.SyncInfo(on_wait=[w], on_update=[])
                    super()._add_instruction(nop)
            super()._add_instruction(inst)

        def _drain_and_barrier(self, tick_clock, wait_clock):
            collector = self.nc.sync.nop(nofuse=True, hint="tail_waits")
            wait_clock.add_sem_waits(
                collector.ins,
                bass_rust.ScopedClock({None: tick_clock.global_clock}),
            )
            si = collector.ins.sync_info
            waits = list(si.on_wait) if si is not None and si.on_wait else []
            if len(waits) > 1:
                si.on_wait = [waits[0]]
                from concourse import mybir as _mybir

                for w in waits[1:]:
                    n = self.nc.sync.nop(nofuse=True, hint="tail_waits")
                    n.ins.sync_info = _mybir.SyncInfo(on_wait=[w], on_update=[])
            self.nc.sync.drain()
            self.nc.all_engine_barrier()
            assert self.sems is not None
            popped = self.nc._tile_sem_poison_stack.pop()
            assert popped is self._sem_poison
            self.nc.clear_and_free_semaphores(
                list(self.sems.allocated().values())
            )
            self.nc.all_engine_barrier()

    return SplitDrainTileContext


def _build_nc(repeat=1):
    import concourse.bass as bass
    import concourse.tile as tile
    import concourse.mybir as mybir
    from concourse.ap import AP

    tile_context_cls = _make_tile_context_cls()
    f32 = mybir.dt.float32
    f16 = mybir.dt.float16
    nc = bass.Bass("TRN2", target_bir_lowering=False, debug=False)
    # per-core x slice: 34 rows (32 owned + halo), zero-padded at edges
    x = nc.dram_tensor("x", [XR * 128, BCC], f32, kind="ExternalInput")
    # wsb[xi, (i, l, j)] = w[i, j, yi_abs(l)*128+xi] for local rows l
    wsb_d = nc.dram_tensor("wsb", [128, 3 * XR * 3], f16,
                           kind="ExternalInput")
    # y stored fp16 (sigmoid output in [0,1]: adds <= ~2.4e-4 abs error)
    # and upcast to f32 on the host -- halves the output DMA traffic
    y = nc.dram_tensor("y", [YR * 128, BCC], f16, kind="ExternalOutput")

    NCH = XR // CL  # 17 x row-chunks
    NGR = XR // G  # 2 T-groups
    TW = 130  # T used width: col c = xi + j, lhsT reads cols 1:129
    # T stored stride: != TW so strided APs never dim-merge, and EVEN so
    # every fp16 block starts 4B-aligned (packed-pair DVE modes need it)
    TWS = 132
    WPITCH = 3 * XR * 3  # wsb row length
    TPITCH = 3 * G * TWS  # T-group tile row length

    with tile_context_cls(nc) as tc:
        with (
            tc.tile_pool(name="cn", bufs=1) as cpool,
            tc.tile_pool(name="xp", bufs=5) as xpool,
            tc.tile_pool(name="op", bufs=3) as opool,
            tc.tile_pool(name="ps", bufs=2, space="PSUM") as ppool,
        ):
            # one-time: weight image, one-hot diagonal masks
            wsb = cpool.tile([128, WPITCH], f16)
            nc.sync.dma_start(out=wsb[:], in_=wsb_d.ap())
            i16 = mybir.dt.int16
            ones = cpool.tile([128, TW], i16)
            nc.gpsimd.memset(ones[:], 1)
            # mask rows stride TWS (even) so each j-row starts 4B-aligned
            masks = cpool.tile([128, 3, TWS], i16)
            for j in range(1, 3):
                # D_j[xi, c] = 1 where c - xi - j == 0
                nc.gpsimd.affine_select(
                    masks[:, j, 0:TW], ones[:],
                    pattern=[[1, TW]], base=-j, channel_multiplier=-1,
                    compare_op=mybir.AluOpType.is_equal, fill=0,
                )
            # T-group tiles live for the whole pass (all 34 rows resident).
            # Two sets, alternating per pass: pass n+1's rebuild would
            # otherwise WAR-wait on pass n's last matmul reads, putting
            # the ~19us group-build chain on the inter-pass critical path
            tsets = [
                [
                    cpool.tile([128, 3, G, TWS], f16, name=f"tg{_s}_{_g}")
                    for _g in range(NGR)
                ]
                for _s in range(2)
            ]
            tgroups = tsets[0]

            xt = {}
            built = set()

            def load_chunk(c):  # noqa: closure rebound per repeat
                if c in xt or c >= NCH:
                    return
                t = xpool.tile([128, CL, BCC], f16, tag="xchunk")
                # 2 MiB reads; SWDGE (gpsimd) casts f32 -> fp16 inline,
                # which the 16-bit PE path needs of the moving side
                src = AP(
                    x.ap().tensor,
                    c * CL * 128 * BCC,
                    [[BCC, 128], [128 * BCC, CL], [1, BCC]],
                )
                nc.gpsimd.dma_start(out=t[:], in_=src)
                xt[c] = t

            def build_group(g):
                if g in built or g >= NGR:
                    return
                built.add(g)
                t = tgroups[g]
                ta = t[:]
                wv = wsb[:]
                for i in range(3):
                    out_i = AP(ta.tensor, ta.offset + i * G * TWS,
                               [[TPITCH, 128], [TWS, G], [1, TW]])

                    def wb(j):
                        return AP(wv.tensor,
                                  wv.offset + i * XR * 3 + g * G * 3 + j,
                                  [[WPITCH, 128], [3, G], [0, TW]])

                    # j=0 placed directly by an affine select (c == xi):
                    # zero-fills the whole block and writes the diagonal in
                    # one gpsimd pass, leaving DVE only the two predicated
                    # copies per dy
                    nc.gpsimd.affine_select(
                        out_i, wb(0),
                        pattern=[[0, G], [1, TW]], base=0,
                        channel_multiplier=-1,
                        compare_op=mybir.AluOpType.is_equal, fill=0.0,
                    )
                    for j in range(1, 3):
                        ma = masks[:, j, :]
                        mb = AP(ma.tensor, ma.offset,
                                [[3 * TWS, 128], [0, G], [1, TW]])
                        nc.vector.copy_predicated(out_i, mb, wb(j))

            for _rep in range(repeat):
              if _rep:
                  xt.clear()
                  built = set()
              tgroups = tsets[_rep % 2]
              # prime: first chunks + the whole (small) T build; MMs start
              # as soon as group 0 and chunk 0 land
              for _c in range(3):
                  load_chunk(_c)
              for _g in range(NGR):
                  build_group(_g)

              ystage = None
              for yo in range(YR):
                  load_chunk((yo + 2) // CL + 1)
                  load_chunk((yo + 2) // CL + 2)
                  if yo % SRW == 0:
                      ystage = opool.tile([128, SRW, BCC], f16, tag="yst")
                  pt = ppool.tile([128, 4, 512], f32, tag="psum")
                  # local input rows l = yo, yo+1, yo+2 (dy = yo+1-l);
                  # the halo padding makes all three always valid
                  for k, l in enumerate((yo, yo + 1, yo + 2)):
                      i_dy = yo + 2 - l
                      tg = tgroups[l // G][:]
                      lo = (i_dy * G + (l % G)) * TWS + 1
                      lhsT = AP(tg.tensor, tg.offset + lo,
                                [[TPITCH, 128], [1, 128]])
                      xc = xt[l // CL][:]
                      # 4 bank-sized batch slices share this lhsT, so the
                      # PE streams 2048 columns per weight load
                      for bk in range(4):
                          rhs = AP(
                              xc.tensor,
                              xc.offset + (l % CL) * BCC + bk * 512,
                              [[CL * BCC, 128], [1, 512]],
                          )
                          nc.tensor.matmul(
                              pt[:, bk, :],
                              lhsT,
                              rhs,
                              start=(k == 0),
                              stop=(k == 2),
                          )
                  # one ACT instruction reads the whole 4-bank PSUM tile
                  # (2048 f32) and writes a full row of fp16: amortizes
                  # the per-instruction ACT overhead 4x
                  ys = ystage[:]
                  dsl = AP(
                      ys.tensor,
                      ys.offset + (yo % SRW) * BCC,
                      [[SRW * BCC, 128], [1, BCC]],
                  )
                  pf = pt[:]
                  nc.scalar.activation(
                      dsl,
                      AP(pf.tensor, pf.offset, [[2048, 128], [1, 2048]]),
                      mybir.ActivationFunctionType.Sigmoid,
                  )
                  if yo % SRW == SRW - 1:
                      dst = AP(
                          y.ap().tensor,
                          (yo - (SRW - 1)) * 128 * BCC,
                          [[BCC, 128], [128 * BCC, SRW], [1, BCC]],
                      )
                      # 2 MiB stores alternating across BOTH HWDGE rings
                      # (SP + ACT): the ~2us HBM write-completion receipt
                      # serializes per ring, so two rings pipeline it
                      eng = nc.scalar if (yo // SRW) % 2 == 0 else nc.sync
                      eng.dma_start(out=dst, in_=ystage[:])
    return nc


def get_nc():
    if "nc" not in _CACHE:
        _CACHE["nc"] = _build_nc()
    return _CACHE["nc"]


def make_in_maps(x, w):
    x = np.asarray(x, dtype=np.float32)
    # x with a zero halo row of the 128x128 grid on each side
    xp = np.zeros((HW + 2 * 128, B), np.float32)
    xp[128 : 128 + HW] = x
    # weight image wg[xi, i, yi, j] = w[i, j, yi*128+xi], halo-padded in yi
    wg = (
        np.asarray(w, dtype=np.float32)
        .reshape(3, 3, H, W)
        .transpose(3, 0, 2, 1)
        .astype(np.float16)
    )  # [xi, i, yi, j]
    wgp = np.zeros((128, 3, H + 2, 3), np.float16)
    wgp[:, :, 1 : 1 + H] = wg
    in_maps = []
    for c in range(NCORES):
        rb, bb = c // NBB, c % NBB
        xs = np.ascontiguousarray(
            xp[rb * YR * 128 : (rb * YR + XR) * 128,
               bb * BCC : (bb + 1) * BCC]
        )
        ws = np.ascontiguousarray(
            wgp[:, :, rb * YR : rb * YR + XR].reshape(128, 3 * XR * 3)
        )
        in_maps.append({"x": xs, "wsb": ws})
    return in_maps


def kernel(x: np.ndarray, w: np.ndarray) -> np.ndarray:
    import time as _time

    from concourse.bass_utils import run_bass_kernel_spmd

    in_maps = make_in_maps(x, w)
    nc = get_nc()
    # The compile hook / remote execution path occasionally fails
    # transiently (observed: a flaky walrus invocation and a recoverable
    # NRT exec error); retry a few times before giving up.
    last_exc = None
    for attempt in range(4):
        try:
            res = run_bass_kernel_spmd(
                nc, in_maps, list(range(NCORES))
            ).results
            break
        except Exception as exc:  # noqa: BLE001
            last_exc = exc
            _time.sleep(2.0 * (attempt + 1))
    else:
        raise last_exc
    out = np.empty((HW, B), np.float32)
    for c in range(NCORES):
        rb, bb = c // NBB, c % NBB
        out[rb * YR * 128 : (rb + 1) * YR * 128,
            bb * BCC : (bb + 1) * BCC] = res[c]["y"]
    return out
